# revision 12
# baseline (speedup 1.0000x reference)
"""Trainium2 Bass kernel for nn_AttentionBlock (B=8, C=128, H=W=64, A=16).

Data-parallel over batch across 8 NeuronCores (one batch each). Per core,
attention over N=4096 pixels, A=16 attention channels:

  xf[C,N] -> q,k [A+1,N] bf16 (17th "bias channel": q row = const g,
  k row = 1, so S' = q^T k = S + g), vT in fp8e5 DoubleRow pair layout.

  Per 512-query chunk, 32 key tiles processed as 8 quads / 16 pairs:
    S'^T quad: 4 concurrent K=17 bf16 matmuls (tile_position row groups)
               into a [128, 1024] PSUM pair tile (double-buffered)
    P = exp(S - 4) in fp8e5, split across two engines per pair:
      ScalarE: activation Exp (bias -4-g) -> float8e5
      DVE:     Schraudolph bits: uint8(min(5.7708*S', 123)) == fp8e5(exp)
    O  += vT2_p^T @ P_p    fp8e5 DoubleRow matmul (256 keys, 0.5 cyc/col)
    Z  += ones^T  @ P_p    fp8e5 DoubleRow matmul (Z replicated on all
                           partitions -> final normalize is elementwise)
  out = O * recip(Z) + (x + bv)   DVE recip+mul, GpSimd residual add
  (bv folded via sum(attn)=1; residual from bf16 x)

fp8 e5m2 throughout the PV path: value range fits (max exp ~2.1e3 <<
57344); bits >= 0x7C are inf/NaN to the PE so the DVE path clamps at 123.
"""

import os
import numpy as np

import concourse.bass as bass
import concourse.mybir as mybir
import concourse.tile as tile
from concourse import bacc
from concourse.bass_utils import run_bass_kernel_spmd

try:
    import ml_dtypes

    _BF16 = np.dtype(ml_dtypes.bfloat16)
except ImportError:  # pragma: no cover
    _BF16 = None

N_CORES = 8
C = 128
A = 16
A1 = A + 1          # +1 bias channel
B = 8
HW = 64
IC = 512            # query-chunk width (one PSUM bank)

A5 = 4.0 / np.log(2.0)          # fp8e5 Schraudolph scale (5.7708)
BSH = 4.0                       # exp shift: P = exp(s - BSH)
GCH = (60.0 - BSH * A5) / A5    # bias-channel constant g (6.3973)
CLIP = 123.0                    # max fp8e5 bits (0x7B = 57344)

# exp-pair engine assignment within a chunk (16 pairs): 1 = ScalarE, 0 = DVE
ENG = [1, 0, 1, 0, 1, 0, 1, 0, 1, 0, 1, 0, 1, 0, 1, 1]


def build_nc(n=4096):
    f32 = mybir.dt.float32
    bf16 = mybir.dt.bfloat16
    fp8 = mybir.dt.float8e5
    u8 = mybir.dt.uint8
    Ident = mybir.ActivationFunctionType.Identity
    Exp = mybir.ActivationFunctionType.Exp
    DR = mybir.MatmulPerfMode.DoubleRow
    Alu = mybir.AluOpType

    nj = n // 128        # 32 key tiles
    npair = nj // 2      # 16
    ni = n // IC         # 8 query chunks
    nx = n // 512        # x chunks

    nc = bacc.Bacc("TRN2", target_bir_lowering=False, debug=False,
                   num_devices=N_CORES)

    xbf_ext = nc.dram_tensor("x_bf", [C, n], bf16, kind="ExternalInput").ap()
    wq4_ext = nc.dram_tensor("wq4", [C, C], bf16, kind="ExternalInput").ap()
    wk4_ext = nc.dram_tensor("wk4", [C, C], bf16, kind="ExternalInput").ap()
    wvT_ext = nc.dram_tensor("wvT", [C, C], bf16, kind="ExternalInput").ap()
    bq4_ext = nc.dram_tensor("bq4", [C, 1], f32, kind="ExternalInput").ap()
    bk4_ext = nc.dram_tensor("bk4", [C, 1], f32, kind="ExternalInput").ap()
    bv_ext = nc.dram_tensor("bv", [C, 1], f32, kind="ExternalInput").ap()
    out_ext = nc.dram_tensor("out", [C, n], f32, kind="ExternalOutput").ap()

    with tile.TileContext(nc) as tc:
        with tc.tile_pool(name="persist", bufs=1) as persist:
            wq4 = persist.tile([C, C], bf16, tag="wq4")
            nc.sync.dma_start(wq4[:], wq4_ext[:])
            wk4 = persist.tile([C, C], bf16, tag="wk4")
            nc.sync.dma_start(wk4[:], wk4_ext[:])
            wvT = persist.tile([C, C], bf16, tag="wvT")
            nc.sync.dma_start(wvT[:], wvT_ext[:])
            bq4_sb = persist.tile([C, 1], f32, tag="bq4_sb")
            nc.sync.dma_start(bq4_sb[:], bq4_ext[:])
            bk4_sb = persist.tile([C, 1], f32, tag="bk4_sb")
            nc.sync.dma_start(bk4_sb[:], bk4_ext[:])
            bv_sb = persist.tile([C, 1], f32, tag="bv_sb")
            nc.sync.dma_start(bv_sb[:], bv_ext[:])

            xf_bf = persist.tile([C, n], bf16, tag="xf_bf")
            for h in range(nx):
                sl = slice(h * 512, (h + 1) * 512)
                nc.gpsimd.dma_start(xf_bf[:, sl], xbf_ext[:, sl])

            ones2 = persist.tile([C, 256], fp8, tag="ones2")
            nc.vector.memset(ones2[:], 1.0)
            zjunk = persist.tile([1, 2], bf16, tag="zjunk")
            nc.vector.memset(zjunk[:], 0.0)
            negb = persist.tile([C, 1], f32, tag="negb")
            nc.vector.memset(negb[:], -(BSH + GCH))

            q4 = persist.tile([C, n], bf16, tag="q4")
            k4 = persist.tile([C, n], bf16, tag="k4")
            vT2 = persist.tile([C, n], fp8, tag="vT2")

            # --- projection phase ---
            # k first (all keys needed by chunk 0), then q chunk-by-chunk,
            # v tiles drained alternately on ScalarE/DVE.
            with tc.tile_pool(name="proj_ps", bufs=3, space="PSUM") as pps:
                for h in range(nx):
                    sl = slice(h * 512, (h + 1) * 512)
                    kp = pps.tile([C, 512], f32, tag="qkp")
                    nc.tensor.matmul(kp[:], wk4[:], xf_bf[:, sl],
                                     start=True, stop=True)
                    nc.vector.tensor_scalar_add(k4[:, sl], kp[:], bk4_sb[:])
                for h in range(nx):
                    sl = slice(h * 512, (h + 1) * 512)
                    qp = pps.tile([C, 512], f32, tag="qkp")
                    nc.tensor.matmul(qp[:], wq4[:], xf_bf[:, sl],
                                     start=True, stop=True)
                    nc.scalar.activation(q4[:, sl], qp[:], Ident,
                                         bias=bq4_sb[:])
                    for jt in (2 * h, 2 * h + 1, 2 * h + 16, 2 * h + 17):
                        vsl = slice(jt * 128, (jt + 1) * 128)
                        vp = pps.tile([C, 128], f32, tag="vp")
                        nc.tensor.matmul(vp[:], xf_bf[:, vsl], wvT[:],
                                         start=True, stop=True)
                        dst = vT2[:, 256 * (jt // 2) + 128 * (jt % 2):
                                  256 * (jt // 2) + 128 * (jt % 2) + 128]
                        if jt % 2 == 0:
                            nc.scalar.activation(dst, vp[:], Ident)
                        else:
                            nc.vector.tensor_copy(dst, vp[:])

            # --- main attention loop ---
            # steps of 3 key tiles (S_ps [128,1536] = 3 banks, double-
            # buffered) like the baseline; exp per step alternates
            # ScalarE/DVE; P lands in a contiguous fp8 arena so PV/Z
            # DoubleRow matmuls consume 2-tile pairs irrespective of step
            # boundaries.
            arena = persist.tile([C, nj * 512], fp8, tag="arena")

            # step s covers tiles [s*3, min(s*3+3, nj))
            nstep = (nj + 2) // 3
            # engine per step: 1 = ScalarE, 0 = DVE (6/5 split per chunk,
            # short last step to DVE)
            seng = [1, 0, 1, 0, 1, 0, 1, 0, 1, 1, 0]

            with tc.tile_pool(name="ep_pool", bufs=2) as epp, \
                 tc.tile_pool(name="ps_S", bufs=2, space="PSUM") as psS, \
                 tc.tile_pool(name="ps_O", bufs=1, space="PSUM") as psO, \
                 tc.tile_pool(name="ps_Z", bufs=1, space="PSUM") as psZ:

                def absorb(O_ps, rd):
                    # 1-column zero matmul whose only job is to carry the
                    # semaphore wait on `rd` (2 bytes of the arena):
                    # the following real matmuls' LDWEIGHTS can then
                    # preload during the wait instead of after it.
                    nc.tensor.matmul(O_ps[0:1, 0:1], zjunk[:, 0:1],
                                     rd.bitcast(bf16), start=False,
                                     stop=False, skip_group_check=True)

                def pv_z(O_ps, Z_ps, p):
                    pt3 = arena[:, 1024 * p:1024 * p + 1024] \
                        .rearrange("p (k x) -> p k x", k=2)
                    v3 = vT2[:, 256 * p:256 * p + 256] \
                        .rearrange("p (k x) -> p k x", k=2)
                    o3 = ones2[:].rearrange("p (k x) -> p k x", k=2)
                    first = p == 0
                    last = p == npair - 1
                    if not first:
                        absorb(O_ps, arena[0:1, 1024 * p + 1022:
                                           1024 * p + 1024])
                    nc.tensor.matmul(O_ps[:], v3, pt3, start=first,
                                     stop=last, perf_mode=DR)
                    nc.tensor.matmul(Z_ps[:], o3, pt3, start=first,
                                     stop=last, perf_mode=DR)

                exp_ends = []  # per emitted exp: its arena end region

                for ic in range(ni):
                    isl = slice(ic * IC, (ic + 1) * IC)
                    O_ps = psO.tile([C, IC], f32, tag="O_ps")
                    Z_ps = psZ.tile([C, IC], f32, tag="Z_ps")
                    done_pairs = 0   # pairs with pv_z issued
                    exp_hist = [0] * (nstep + 1)  # tiles exp'd after step s
                    for s in range(nstep):
                        j0 = 3 * s
                        tl = min(3, nj - j0)
                        # absorber for the S_ps WAR wait (exp two steps
                        # back, possibly in the previous chunk)
                        if len(exp_ends) >= 2:
                            absorb(O_ps, exp_ends[-2])
                        S_ps = psS.tile([128, 1536], f32, tag="S_ps")
                        for r in range(tl):
                            jt = j0 + r
                            p0 = 32 * r
                            nc.tensor.matmul(
                                S_ps[:, r * 512:r * 512 + 512],
                                k4[p0:p0 + A1, jt * 128:(jt + 1) * 128],
                                q4[p0:p0 + A1, isl],
                                start=True, stop=True,
                                tile_position=(p0, 0))
                        # flush pairs completed TWO steps back: when
                        # exp(s-2) retires, S(s) is the next TensorE item
                        # (not these PVs), so the exp->S->exp chain stays
                        # tight.
                        if s >= 2:
                            while done_pairs < exp_hist[s - 2] // 2:
                                pv_z(O_ps, Z_ps, done_pairs)
                                done_pairs += 1
                        dst = arena[:, j0 * 512:(j0 + tl) * 512]
                        if seng[s]:
                            nc.scalar.activation(dst, S_ps[:, :tl * 512],
                                                 Exp, bias=negb[:])
                        else:
                            nc.vector.tensor_scalar(
                                dst.bitcast(u8), S_ps[:, :tl * 512],
                                A5, CLIP, Alu.mult, Alu.min)
                        exp_ends.append(
                            arena[0:1, (j0 + tl) * 512 - 2:(j0 + tl) * 512])
                        exp_hist[s] = (3 * s + tl if s == 0 else
                                       exp_hist[s - 1] + tl)
                    while done_pairs < npair:
                        pv_z(O_ps, Z_ps, done_pairs)
                        done_pairs += 1

                    recip = epp.tile([C, IC], f32, tag="recip")
                    nc.vector.reciprocal_approx_fast(recip[:], Z_ps[:])
                    xr = epp.tile([C, IC], bf16, tag="xr")
                    nc.scalar.activation(xr[:], xf_bf[:, isl], Ident,
                                         bias=bv_sb[:])
                    o1 = epp.tile([C, IC], f32, tag="o1")
                    nc.vector.tensor_mul(o1[:], O_ps[:], recip[:])
                    o2 = epp.tile([C, IC], f32, tag="o2")
                    nc.gpsimd.tensor_add(o2[:], o1[:], xr[:])
                    nc.sync.dma_start(out_ext[:, isl], o2[:])

    nc.compile()
    return nc


_NC_CACHE = {}


def _get_nc(n=4096):
    if n not in _NC_CACHE:
        _NC_CACHE[n] = build_nc(n)
    return _NC_CACHE[n]


def _spread(w):
    """[A, C] weight -> [C, C] lhsT with W.T in 4 row-group column bands
    (17th column of each band = 0: the bias channel comes from the bias)."""
    out = np.zeros((C, C), dtype=np.float32)
    for r in range(4):
        out[:, 32 * r:32 * r + A] = w.T
    return out.astype(_BF16)


def _spread_bias(b, ch):
    out = np.zeros((C, 1), dtype=np.float32)
    for r in range(4):
        out[32 * r:32 * r + A, 0] = b
        out[32 * r + A, 0] = ch
    return out


def kernel(x, Wq, bq, Wk, bk, Wv, bv):
    x = np.asarray(x, dtype=np.float32)
    Wq = np.asarray(Wq, dtype=np.float32)
    bq = np.asarray(bq, dtype=np.float32)
    Wk = np.asarray(Wk, dtype=np.float32)
    bk = np.asarray(bk, dtype=np.float32)
    Wv = np.asarray(Wv, dtype=np.float32)
    bv = np.asarray(bv, dtype=np.float32)

    b, c, hh, ww = x.shape
    n = hh * ww
    assert (b, c) == (B, C) and n == 4096

    nc = _get_nc(n)

    in_common = {
        "wq4": _spread(Wq),
        "wk4": _spread(Wk),
        "wvT": np.ascontiguousarray(Wv.T).astype(_BF16),
        "bq4": _spread_bias(bq, GCH),
        "bk4": _spread_bias(bk, 1.0),
        "bv": np.ascontiguousarray(bv.reshape(C, 1)),
    }
    in_maps = []
    for i in range(B):
        xi = np.ascontiguousarray(x[i].reshape(C, n))
        in_maps.append({"x_bf": xi.astype(_BF16), **in_common})

    trace = bool(int(os.environ.get("BASS_KERNEL_PROFILE", "0")))
    res = run_bass_kernel_spmd(nc, in_maps, core_ids=list(range(N_CORES)),
                               trace=trace)
    if trace:
        kernel.last_exec_time_ns = res.exec_time_ns
        kernel.last_results = res

    out = np.stack([res.results[i]["out"].reshape(C, hh, ww)
                    for i in range(B)])
    return out


# revision 13
# speedup vs baseline: 1.1481x; 1.1481x over previous
"""Trainium2 Bass kernel for nn_AttentionBlock (B=8, C=128, H=W=64, A=16).

Data-parallel over batch across 8 NeuronCores (one batch each). Per core,
attention over N=4096 pixels, A=16 attention channels:

  xf[C,N] -> q,k [A+1,N] bf16 (17th "bias channel": q row = const g,
  k row = 1, so S' = q^T k = S + g), vT in fp8e5 DoubleRow pair layout.

  Per 512-query chunk, 32 key tiles processed as 8 quads / 16 pairs:
    S'^T quad: 4 concurrent K=17 bf16 matmuls (tile_position row groups)
               into a [128, 1024] PSUM pair tile (double-buffered)
    P = exp(S - 4) in fp8e5, split across two engines per pair:
      ScalarE: activation Exp (bias -4-g) -> float8e5
      DVE:     Schraudolph bits: uint8(min(5.7708*S', 123)) == fp8e5(exp)
    O  += vT2_p^T @ P_p    fp8e5 DoubleRow matmul (256 keys, 0.5 cyc/col)
    Z  += ones^T  @ P_p    fp8e5 DoubleRow matmul (Z replicated on all
                           partitions -> final normalize is elementwise)
  out = O * recip(Z) + (x + bv)   DVE recip+mul, GpSimd residual add
  (bv folded via sum(attn)=1; residual from bf16 x)

fp8 e5m2 throughout the PV path: value range fits (max exp ~2.1e3 <<
57344); bits >= 0x7C are inf/NaN to the PE so the DVE path clamps at 123.
"""

import os
import numpy as np

import concourse.bass as bass
import concourse.mybir as mybir
import concourse.tile as tile
from concourse import bacc
from concourse.bass_utils import run_bass_kernel_spmd

try:
    import ml_dtypes

    _BF16 = np.dtype(ml_dtypes.bfloat16)
except ImportError:  # pragma: no cover
    _BF16 = None

N_CORES = 8
C = 128
A = 16
A1 = A + 1          # +1 bias channel
B = 8
HW = 64
IC = 512            # query-chunk width (one PSUM bank)

A5 = 4.0 / np.log(2.0)          # fp8e5 Schraudolph scale (5.7708)
BSH = 4.0                       # exp shift: P = exp(s - BSH)
GCH = (60.0 - BSH * A5) / A5    # bias-channel constant g (6.3973)
CLIP = 123.0                    # max fp8e5 bits (0x7B = 57344)

# exp-pair engine assignment within a chunk (16 pairs): 1 = ScalarE, 0 = DVE
ENG = [1, 0, 1, 0, 1, 0, 1, 0, 1, 0, 1, 0, 1, 0, 1, 1]


def build_nc(n=4096):
    f32 = mybir.dt.float32
    bf16 = mybir.dt.bfloat16
    fp8 = mybir.dt.float8e5
    u8 = mybir.dt.uint8
    Ident = mybir.ActivationFunctionType.Identity
    Exp = mybir.ActivationFunctionType.Exp
    DR = mybir.MatmulPerfMode.DoubleRow
    Alu = mybir.AluOpType

    nj = n // 128        # 32 key tiles
    npair = nj // 2      # 16
    ni = n // IC         # 8 query chunks
    nx = n // 512        # x chunks

    nc = bacc.Bacc("TRN2", target_bir_lowering=False, debug=False,
                   num_devices=N_CORES)

    xbf_ext = nc.dram_tensor("x_bf", [C, n], bf16, kind="ExternalInput").ap()
    wq4_ext = nc.dram_tensor("wq4", [C, C], bf16, kind="ExternalInput").ap()
    wk4_ext = nc.dram_tensor("wk4", [C, C], bf16, kind="ExternalInput").ap()
    wvT_ext = nc.dram_tensor("wvT", [C, C], bf16, kind="ExternalInput").ap()
    bq4_ext = nc.dram_tensor("bq4", [C, 1], f32, kind="ExternalInput").ap()
    bk4_ext = nc.dram_tensor("bk4", [C, 1], f32, kind="ExternalInput").ap()
    bv_ext = nc.dram_tensor("bv", [C, 1], f32, kind="ExternalInput").ap()
    out_ext = nc.dram_tensor("out", [C, n], f32, kind="ExternalOutput").ap()

    with tile.TileContext(nc) as tc:
        with tc.tile_pool(name="persist", bufs=1) as persist:
            wq4 = persist.tile([C, C], bf16, tag="wq4")
            nc.sync.dma_start(wq4[:], wq4_ext[:])
            wk4 = persist.tile([C, C], bf16, tag="wk4")
            nc.sync.dma_start(wk4[:], wk4_ext[:])
            wvT = persist.tile([C, C], bf16, tag="wvT")
            nc.sync.dma_start(wvT[:], wvT_ext[:])
            bq4_sb = persist.tile([C, 1], f32, tag="bq4_sb")
            nc.sync.dma_start(bq4_sb[:], bq4_ext[:])
            bk4_sb = persist.tile([C, 1], f32, tag="bk4_sb")
            nc.sync.dma_start(bk4_sb[:], bk4_ext[:])
            bv_sb = persist.tile([C, 1], f32, tag="bv_sb")
            nc.sync.dma_start(bv_sb[:], bv_ext[:])

            xf_bf = persist.tile([C, n], bf16, tag="xf_bf")
            for h in range(nx):
                sl = slice(h * 512, (h + 1) * 512)
                nc.gpsimd.dma_start(xf_bf[:, sl], xbf_ext[:, sl])

            ones2 = persist.tile([C, 256], fp8, tag="ones2")
            nc.vector.memset(ones2[:], 1.0)
            zjunk = persist.tile([1, 2], bf16, tag="zjunk")
            nc.vector.memset(zjunk[:], 0.0)
            negb = persist.tile([C, 1], f32, tag="negb")
            nc.vector.memset(negb[:], -(BSH + GCH))

            q4 = persist.tile([C, n], bf16, tag="q4")
            k4 = persist.tile([C, n], bf16, tag="k4")
            vT2 = persist.tile([C, n], fp8, tag="vT2")

            # --- projection phase ---
            # k first (all keys needed by chunk 0), then q chunk-by-chunk,
            # v tiles drained alternately on ScalarE/DVE.
            with tc.tile_pool(name="proj_ps", bufs=3, space="PSUM") as pps:
                for h in range(nx):
                    sl = slice(h * 512, (h + 1) * 512)
                    kp = pps.tile([C, 512], f32, tag="qkp")
                    nc.tensor.matmul(kp[:], wk4[:], xf_bf[:, sl],
                                     start=True, stop=True)
                    nc.vector.tensor_scalar_add(k4[:, sl], kp[:], bk4_sb[:])
                for h in range(nx):
                    sl = slice(h * 512, (h + 1) * 512)
                    qp = pps.tile([C, 512], f32, tag="qkp")
                    nc.tensor.matmul(qp[:], wq4[:], xf_bf[:, sl],
                                     start=True, stop=True)
                    nc.scalar.activation(q4[:, sl], qp[:], Ident,
                                         bias=bq4_sb[:])
                    for jt in (2 * h, 2 * h + 1, 2 * h + 16, 2 * h + 17):
                        vsl = slice(jt * 128, (jt + 1) * 128)
                        vp = pps.tile([C, 128], f32, tag="vp")
                        nc.tensor.matmul(vp[:], xf_bf[:, vsl], wvT[:],
                                         start=True, stop=True)
                        dst = vT2[:, 256 * (jt // 2) + 128 * (jt % 2):
                                  256 * (jt // 2) + 128 * (jt % 2) + 128]
                        if jt % 2 == 0:
                            nc.scalar.activation(dst, vp[:], Ident)
                        else:
                            nc.vector.tensor_copy(dst, vp[:])

            # --- main attention loop ---
            # steps of 3 key tiles (S_ps [128,1536] = 3 banks, double-
            # buffered) like the baseline; exp per step alternates
            # ScalarE/DVE; P lands in a contiguous fp8 arena so PV/Z
            # DoubleRow matmuls consume 2-tile pairs irrespective of step
            # boundaries.
            arena = persist.tile([C, nj * 512], fp8, tag="arena")

            # pair-step s covers tiles [2s, 2s+2); S_ps [128,1024]
            # x3 buffers -> the exp(p-3) -> S(p) PSUM-reuse chain has two
            # steps of slack and never binds; PV/Z flushed with lag 3 so
            # their exp waits are pre-satisfied (no LDWEIGHTS ramp).
            # engine per pair: 1 = ScalarE, 0 = DVE (9/7 split)
            seng = [1, 0, 1, 0, 1, 0, 1, 0, 1, 0, 1, 0, 1, 0, 1, 1]
            LAG = int(os.environ.get("BASS_LAG", "3"))

            with tc.tile_pool(name="ep_pool", bufs=2) as epp, \
                 tc.tile_pool(name="ps_S", bufs=3, space="PSUM") as psS, \
                 tc.tile_pool(name="ps_O", bufs=1, space="PSUM") as psO, \
                 tc.tile_pool(name="ps_Z", bufs=1, space="PSUM") as psZ:

                def pv_z(O_ps, Z_ps, p):
                    pt3 = arena[:, 1024 * p:1024 * p + 1024] \
                        .rearrange("p (k x) -> p k x", k=2)
                    v3 = vT2[:, 256 * p:256 * p + 256] \
                        .rearrange("p (k x) -> p k x", k=2)
                    o3 = ones2[:].rearrange("p (k x) -> p k x", k=2)
                    first = p == 0
                    last = p == npair - 1
                    nc.tensor.matmul(O_ps[:], v3, pt3, start=first,
                                     stop=last, perf_mode=DR)
                    nc.tensor.matmul(Z_ps[:], o3, pt3, start=first,
                                     stop=last, perf_mode=DR)

                for ic in range(ni):
                    isl = slice(ic * IC, (ic + 1) * IC)
                    O_ps = psO.tile([C, IC], f32, tag="O_ps")
                    Z_ps = psZ.tile([C, IC], f32, tag="Z_ps")
                    done = 0
                    for p in range(npair):
                        S_ps = psS.tile([128, 1024], f32, tag="S_ps")
                        rgb = (p % 2) * 2   # alternate row-group pairs
                        for r in range(2):
                            jt = 2 * p + r
                            p0 = 32 * (rgb + r)
                            nc.tensor.matmul(
                                S_ps[:, r * 512:r * 512 + 512],
                                k4[p0:p0 + A1, jt * 128:(jt + 1) * 128],
                                q4[p0:p0 + A1, isl],
                                start=True, stop=True,
                                tile_position=(p0, 0))
                        while done <= p - LAG:
                            pv_z(O_ps, Z_ps, done)
                            done += 1
                        dst = arena[:, 1024 * p:1024 * p + 1024]
                        if seng[p]:
                            nc.scalar.activation(dst, S_ps[:], Exp,
                                                 bias=negb[:])
                        else:
                            nc.vector.tensor_scalar(
                                dst.bitcast(u8), S_ps[:],
                                A5, CLIP, Alu.mult, Alu.min)
                    while done < npair:
                        pv_z(O_ps, Z_ps, done)
                        done += 1

                    recip = epp.tile([C, IC], f32, tag="recip")
                    nc.vector.reciprocal_approx_fast(recip[:], Z_ps[:])
                    xr = epp.tile([C, IC], bf16, tag="xr")
                    nc.scalar.activation(xr[:], xf_bf[:, isl], Ident,
                                         bias=bv_sb[:])
                    o1 = epp.tile([C, IC], f32, tag="o1")
                    nc.vector.tensor_mul(o1[:], O_ps[:], recip[:])
                    o2 = epp.tile([C, IC], f32, tag="o2")
                    nc.gpsimd.tensor_add(o2[:], o1[:], xr[:])
                    nc.sync.dma_start(out_ext[:, isl], o2[:])

    nc.compile()
    return nc


_NC_CACHE = {}


def _get_nc(n=4096):
    if n not in _NC_CACHE:
        _NC_CACHE[n] = build_nc(n)
    return _NC_CACHE[n]


def _spread(w):
    """[A, C] weight -> [C, C] lhsT with W.T in 4 row-group column bands
    (17th column of each band = 0: the bias channel comes from the bias)."""
    out = np.zeros((C, C), dtype=np.float32)
    for r in range(4):
        out[:, 32 * r:32 * r + A] = w.T
    return out.astype(_BF16)


def _spread_bias(b, ch):
    out = np.zeros((C, 1), dtype=np.float32)
    for r in range(4):
        out[32 * r:32 * r + A, 0] = b
        out[32 * r + A, 0] = ch
    return out


def kernel(x, Wq, bq, Wk, bk, Wv, bv):
    x = np.asarray(x, dtype=np.float32)
    Wq = np.asarray(Wq, dtype=np.float32)
    bq = np.asarray(bq, dtype=np.float32)
    Wk = np.asarray(Wk, dtype=np.float32)
    bk = np.asarray(bk, dtype=np.float32)
    Wv = np.asarray(Wv, dtype=np.float32)
    bv = np.asarray(bv, dtype=np.float32)

    b, c, hh, ww = x.shape
    n = hh * ww
    assert (b, c) == (B, C) and n == 4096

    nc = _get_nc(n)

    in_common = {
        "wq4": _spread(Wq),
        "wk4": _spread(Wk),
        "wvT": np.ascontiguousarray(Wv.T).astype(_BF16),
        "bq4": _spread_bias(bq, GCH),
        "bk4": _spread_bias(bk, 1.0),
        "bv": np.ascontiguousarray(bv.reshape(C, 1)),
    }
    in_maps = []
    for i in range(B):
        xi = np.ascontiguousarray(x[i].reshape(C, n))
        in_maps.append({"x_bf": xi.astype(_BF16), **in_common})

    trace = bool(int(os.environ.get("BASS_KERNEL_PROFILE", "0")))
    res = run_bass_kernel_spmd(nc, in_maps, core_ids=list(range(N_CORES)),
                               trace=trace)
    if trace:
        kernel.last_exec_time_ns = res.exec_time_ns
        kernel.last_results = res

    out = np.stack([res.results[i]["out"].reshape(C, hh, ww)
                    for i in range(B)])
    return out
